# revision 20
# baseline (speedup 1.0000x reference)
"""Trainium2 Bass kernel for nn_Attention_module_52166672777937.

Data-parallel over batch across 8 NeuronCores (4 sequences per core).

Algorithmic restructuring (numerically validated against the reference):
the module only consumes the attention output at the LAST valid position
of each sequence (take_along_axis with lengths-1), and attention is
causal, so only ONE query row per sequence matters.  Consequences:

  * q is computed for a single position per sequence.
  * K is never materialized: scores = (qblk.T @ Wk) @ x.T, using
    associativity of the K projection with the score contraction.
  * softmax runs over [H=8, L] scores per sequence (no L x L matrix).
  * ctx = softmax(scores) @ V needs V = x @ Wv.T for all positions -- the
    dominant matmul, kept on TensorE at fp32r full rate.

Device layout: x is built in transposed [E, L] layout directly via a
one-hot matmul gather (onehot[c, l] = (data[l] == c), x.T = emb.T @
onehot + pe.T), which feeds both the score matmul and the V projection
without any transposes of large tensors.
"""

import math
import sys

import ml_dtypes
import numpy as np

sys.path.insert(0, "/opt/trn_rl_repo")

import concourse.bacc as bacc
import concourse.bass as bass
import concourse.mybir as mybir
import concourse.tile as tile
from concourse.bass_utils import run_bass_kernel_spmd

dt = mybir.dt
AF = mybir.ActivationFunctionType
ALU = mybir.AluOpType
PSUM = bass.MemorySpace.PSUM

N_CORES = 8
B, L = 32, 1000
LP = 1024                 # padded sequence length (2 x 512 column tiles)
TW = 512                  # column-tile width (max fp32 moving operand / PSUM bank)
NT = LP // TW             # column tiles per sequence
BPC = B // N_CORES        # sequences per core
NCH = 256                 # vocabulary
E = 512                   # embedding dim
D = 512                   # d_model
NH, DH = 8, 64            # heads
HS = 512                  # pred hidden size
NOUT = 8
NEG = -1.0e30
SCALE = 1.0 / math.sqrt(DH)


def _build():
    nc = bacc.Bacc(
        "TRN2", target_bir_lowering=False, debug=False, num_devices=N_CORES
    )

    f32 = dt.float32
    f32r = dt.float32r
    bf16 = dt.bfloat16
    # --- packed inputs (few wide DMAs instead of many narrow ones) ------
    # bf16s: data+idxlast row; emb; wqT|wk|wvT; peT; id8b
    d_drow = nc.dram_tensor("drow", [1, BPC * LP + BPC], bf16,
                            kind="ExternalInput")
    d_emb = nc.dram_tensor("emb", [NCH, E], bf16, kind="ExternalInput")
    d_wb = nc.dram_tensor("wb", [E, 3 * D], bf16, kind="ExternalInput")
    d_peT = nc.dram_tensor("peT", [E, LP], bf16, kind="ExternalInput")
    d_id8b = nc.dram_tensor("id8b", [NH, NH], bf16, kind="ExternalInput")
    # f32r: w1T|w2T ; ones8
    d_wr = nc.dram_tensor("wr", [D, HS + NOUT], f32r, kind="ExternalInput")
    d_ones8 = nc.dram_tensor("ones8", [NH, 1], f32r, kind="ExternalInput")
    # f32: pelastT|hmask (512-row); bq|b1|cvals (128-row); plast|b2|iota (8-row)
    d_fA = nc.dram_tensor("fA", [D, BPC + NH], f32, kind="ExternalInput")
    d_fB = nc.dram_tensor("fB", [128, 10], f32, kind="ExternalInput")
    d_fC = nc.dram_tensor("fC", [NH, BPC + 1 + LP], f32, kind="ExternalInput")
    d_out = nc.dram_tensor("out", [1, BPC], f32, kind="ExternalOutput")

    with tile.TileContext(nc) as tc:
        with (
            tc.tile_pool(name="const", bufs=1) as cp,
            tc.tile_pool(name="work", bufs=2) as wp,
            tc.tile_pool(name="psx", bufs=2, space=PSUM) as psx,
            tc.tile_pool(name="psv", bufs=2, space=PSUM) as psv,
            tc.tile_pool(name="pss", bufs=2, space=PSUM) as pss,
            tc.tile_pool(name="psc", bufs=2, space=PSUM) as psc,
        ):
            # ---------------- constant loads (packed) -------------------
            # critical path first: data row, broadcasts, emb, small consts
            dbb = []
            for b in range(BPC):
                t = cp.tile([128, LP], bf16, name=f"db{b}", tag=f"db{b}")
                nc.sync.dma_start(
                    out=t[:],
                    in_=d_drow[:, b * LP:(b + 1) * LP].to_broadcast(
                        (128, LP)),
                )
                dbb.append(t)
            drow_sb = cp.tile([1, BPC * LP + BPC], bf16, name="drow",
                              tag="drow")
            nc.sync.dma_start(out=drow_sb[:], in_=d_drow[:])
            data_row = drow_sb[:, 0:BPC * LP]
            idxb_sb = cp.tile([128, BPC], bf16, name="idxb", tag="idxb")
            nc.sync.dma_start(
                out=idxb_sb[:],
                in_=d_drow[:, BPC * LP:].to_broadcast((128, BPC)),
            )
            embp_sb = cp.tile([128, 2, E], bf16, name="embp", tag="embp")
            nc.sync.dma_start(
                out=embp_sb[:],
                in_=d_emb[:].rearrange("(c p) n -> p c n", p=128),
            )
            emb_sb = [embp_sb[:, c, :] for c in range(2)]
            fB_sb = cp.tile([128, 10], f32, name="fB", tag="fB")
            nc.sync.dma_start(out=fB_sb[:], in_=d_fB[:])
            bq_sb = fB_sb[:, 0:4]
            b1_sb = fB_sb[:, 4:8]
            cvals_sb = fB_sb[:, 8:10]
            fC_sb = cp.tile([NH, BPC + 1 + LP], f32, name="fC", tag="fC")
            nc.sync.dma_start(out=fC_sb[:], in_=d_fC[:])
            plast_sb = fC_sb[:, 0:BPC]
            b2_sb = fC_sb[:, BPC:BPC + 1]
            iota_sb = fC_sb[:, BPC + 1:]
            fA_sb = cp.tile([128, 4, BPC + NH], f32, name="fA", tag="fA")
            nc.sync.dma_start(
                out=fA_sb[:], in_=d_fA[:].rearrange("(c p) n -> p c n", p=128)
            )
            pelT_sb = [fA_sb[:, m, 0:BPC] for m in range(4)]
            hmask_sb = [fA_sb[:, m, BPC:BPC + NH] for m in range(4)]
            id8b_sb = cp.tile([NH, NH], bf16, name="id8b", tag="id8b")
            nc.sync.dma_start(out=id8b_sb[:], in_=d_id8b[:])
            ones8_sb = cp.tile([NH, 1], f32r, name="ones8", tag="ones8")
            nc.sync.dma_start(out=ones8_sb[:], in_=d_ones8[:])
            # bulk weights on the gpsimd queue (issues in parallel)
            wb_sb = cp.tile([128, 4, 3 * D], bf16, name="wb", tag="wb")
            nc.sync.dma_start(
                out=wb_sb[:], in_=d_wb[:].rearrange("(c p) n -> p c n", p=128)
            )
            wqT_sb = [wb_sb[:, e, 0:D] for e in range(4)]
            wk_sb = [wb_sb[:, c, D:2 * D] for c in range(4)]
            wvT_sb = [wb_sb[:, e, 2 * D:3 * D] for e in range(4)]
            peTp_sb = cp.tile([128, 4, LP], bf16, name="peTp", tag="peTp")
            nc.sync.dma_start(
                out=peTp_sb[:],
                in_=d_peT[:].rearrange("(c p) n -> p c n", p=128),
            )
            peT_sb = [peTp_sb[:, e, :] for e in range(4)]
            wr_sb = cp.tile([128, 4, HS + NOUT], f32r, name="wr", tag="wr")
            nc.sync.dma_start(
                out=wr_sb[:], in_=d_wr[:].rearrange("(c p) n -> p c n", p=128)
            )
            w1T_sb = [wr_sb[:, m, 0:HS] for m in range(4)]
            w2T_sb = [wr_sb[:, m, HS:HS + NOUT] for m in range(4)]

            # causal/validity masks, additive (0 valid / -1e30 invalid)
            madd_sb = []
            for b in range(BPC):
                m8 = cp.tile([NH, LP], f32, name=f"madd{b}", tag=f"madd{b}")
                nc.vector.tensor_scalar(
                    m8[:], iota_sb[:], plast_sb[:, b:b + 1], NEG,
                    ALU.is_gt, ALU.mult,
                )
                madd_sb.append(m8)

            # ---------------- x_last gather -> q ------------------------
            ohl = []
            for c in range(2):
                t = cp.tile([128, BPC], bf16, name=f"ohl{c}", tag=f"ohl{c}")
                nc.vector.tensor_scalar(
                    t[:], idxb_sb[:], cvals_sb[:, c:c + 1], None, ALU.is_equal
                )
                ohl.append(t)
            # x_last.T [E, BPC] = emb.T @ onehot_last + pe_last.T
            xlast_sb = []
            for e in range(4):
                p = psx.tile([128, BPC], f32, name=f"xlp{e}", tag="xtp")
                for c in range(2):
                    nc.tensor.matmul(
                        p[:], (emb_sb[c][:, e * 128:(e + 1) * 128]),
                        (ohl[c][:]), start=(c == 0), stop=(c == 1),
                    )
                t = cp.tile([128, BPC], bf16, name=f"xlast{e}", tag=f"xlast{e}")
                nc.vector.tensor_tensor(t[:], p[:], pelT_sb[e][:], ALU.add)
                xlast_sb.append(t)
            # q.T [D, BPC] = Wq @ x_last.T + bq
            qT_sb = []
            for d in range(4):
                p = psv.tile([128, BPC], f32, name=f"qp{d}", tag="vp")
                for e in range(4):
                    nc.tensor.matmul(
                        p[:], (wqT_sb[e][:, d * 128:(d + 1) * 128]),
                        (xlast_sb[e][:]), start=(e == 0), stop=(e == 3),
                    )
                t = cp.tile([128, BPC], f32, name=f"qT{d}", tag=f"qT{d}")
                nc.vector.tensor_scalar(t[:], p[:], bq_sb[:, d:d + 1], None,
                                        ALU.add)
                qT_sb.append(t)

            # ---------------- main loop over sequences ------------------
            out_sb = cp.tile([1, BPC], f32, name="out_sb", tag="out_sb")
            ctxT_sb = [cp.tile([128, BPC], f32r, name=f"ctxT{m}", tag=f"ctxT{m}")
                       for m in range(4)]
            for b in range(BPC):
                # --- per-sequence qkvec = qblk.T @ Wk (K never formed) --
                qblk = []
                for d in range(4):
                    t = cp.tile([128, NH], bf16, name=f"qblk{b}_{d}",
                                tag=f"qblk{b}_{d}")
                    nc.vector.tensor_scalar(
                        t[:], hmask_sb[d][:], qT_sb[d][:, b:b + 1], None,
                        ALU.mult,
                    )
                    qblk.append(t)
                qkvp = pss.tile([NH, E], f32, name=f"qkvp{b}", tag="sp")
                for d in range(4):
                    nc.tensor.matmul(
                        qkvp[:], (qblk[d][:]), (wk_sb[d][:]),
                        start=(d == 0), stop=(d == 3),
                    )
                qkv_sb = wp.tile([NH, E], bf16, name=f"qkv{b}", tag="qkv",
                                 bufs=2)
                nc.vector.tensor_copy(qkv_sb[:], qkvp[:])
                qkvT = []
                for e in range(4):
                    tp = pss.tile([128, NH], bf16, name=f"qkvTp{b}_{e}",
                                  tag="sp")
                    nc.tensor.transpose(
                        tp[:], qkv_sb[:, e * 128:(e + 1) * 128], id8b_sb[:]
                    )
                    t = cp.tile([128, NH], bf16, name=f"qkvT{b}_{e}",
                                tag=f"qkvT{b}_{e}")
                    nc.vector.tensor_copy(t[:], tp[:])
                    qkvT.append(t)

                # --- attention over the sequence ------------------------
                ctxp = psc.tile([NH, D], f32, name=f"ctx{b}", tag="cp")
                den_sb = wp.tile([NH, NT], f32, name=f"den{b}", tag="den",
                                 bufs=2)
                for t in range(NT):
                    col0 = b * LP + t * TW
                    # one-hot [256, TW] for this tile's positions
                    oh = []
                    for c in range(2):
                        o = wp.tile([128, TW], bf16, name=f"oh{b}_{t}_{c}",
                                    tag="oh", bufs=6)
                        nc.vector.tensor_scalar(
                            o[:], dbb[b][:, t * TW:(t + 1) * TW],
                            cvals_sb[:, c:c + 1], None, ALU.is_equal,
                        )
                        oh.append(o)
                    # x.T tile [E, TW] = emb.T @ onehot + pe.T
                    xT = []
                    for e in range(4):
                        p = psx.tile([128, TW], f32, name=f"xtp{b}_{t}_{e}",
                                     tag="xtp")
                        for c in range(2):
                            nc.tensor.matmul(
                                p[:],
                                (emb_sb[c][:, e * 128:(e + 1) * 128]),
                                (oh[c][:]), start=(c == 0), stop=(c == 1),
                            )
                        x = wp.tile([128, TW], bf16, name=f"xT{b}_{t}_{e}",
                                    tag=f"xT{e}", bufs=3)
                        nc.vector.tensor_tensor(
                            x[:], p[:], peT_sb[e][:, t * TW:(t + 1) * TW],
                            ALU.add,
                        )
                        xT.append(x)
                    # scores [NH, TW] = qkvec @ x.T
                    sp = pss.tile([NH, TW], f32, name=f"s{b}_{t}", tag="sp")
                    for e in range(4):
                        nc.tensor.matmul(
                            sp[:], (qkvT[e][:]), (xT[e][:]),
                            start=(e == 0), stop=(e == 3),
                        )
                    # masked scaled scores -> exp (+ row-sum for denom)
                    sm = wp.tile([NH, TW], f32, name=f"sm{b}_{t}", tag="sm",
                                 bufs=3)
                    nc.vector.scalar_tensor_tensor(
                        sm[:], sp[:], SCALE, madd_sb[b][:, t * TW:(t + 1) * TW],
                        ALU.mult, ALU.add,
                    )
                    ex = wp.tile([NH, TW], bf16, name=f"ex{b}_{t}", tag="ex",
                                 bufs=3)
                    nc.scalar.activation(
                        ex[:], sm[:], AF.Exp,
                        accum_out=den_sb[:, t:t + 1],
                    )
                    # V tile + ctx accumulation, one 128-row block at a time
                    for lc in range(4):
                        ap = pss.tile([128, NH], bf16, name=f"aTp{b}_{t}_{lc}",
                                      tag="sp")
                        nc.tensor.transpose(
                            ap[:], ex[:, lc * 128:(lc + 1) * 128], id8b_sb[:]
                        )
                        aT = wp.tile([128, NH], bf16, name=f"aT{b}_{t}_{lc}",
                                     tag="aT", bufs=8)
                        nc.scalar.copy(aT[:], ap[:])
                        vp = psv.tile([128, D], f32, name=f"vp{b}_{t}_{lc}",
                                      tag="vp")
                        for e in range(4):
                            nc.tensor.matmul(
                                vp[:],
                                (xT[e][:, lc * 128:(lc + 1) * 128]),
                                (wvT_sb[e][:]),
                                start=(e == 0), stop=(e == 3),
                            )
                        v = wp.tile([128, D], bf16, name=f"v{b}_{t}_{lc}",
                                    tag="v", bufs=4)
                        if lc % 2 == 0:
                            nc.vector.tensor_copy(v[:], vp[:])
                        else:
                            nc.scalar.copy(v[:], vp[:])
                        nc.tensor.matmul(
                            ctxp[:], (aT[:]), (v[:]),
                            start=(t == 0 and lc == 0),
                            stop=(t == NT - 1 and lc == 3),
                        )
                # normalize ctx rows by the masked softmax denominator
                dsum = wp.tile([NH, 1], f32, name=f"dsum{b}", tag="dsum",
                               bufs=2)
                nc.vector.reduce_sum(dsum[:], den_sb[:],
                                     axis=mybir.AxisListType.X)
                rec = wp.tile([NH, 1], f32, name=f"rec{b}", tag="rec", bufs=2)
                nc.vector.reciprocal(rec[:], dsum[:])
                ctx_sb = wp.tile([NH, D], bf16, name=f"ctxs{b}", tag="ctxs",
                                 bufs=2)
                nc.scalar.activation(ctx_sb[:], ctxp[:], AF.Copy,
                                     scale=rec[:])
                # extract block-diagonal -> ctx.T [D, BPC] column b
                for m in range(4):
                    tp = pss.tile([128, NH], bf16, name=f"ctp{b}_{m}", tag="sp")
                    nc.tensor.transpose(
                        tp[:], ctx_sb[:, m * 128:(m + 1) * 128], id8b_sb[:]
                    )
                    scr = wp.tile([128, NH], f32, name=f"scr{b}_{m}",
                                  tag="scr", bufs=2)
                    nc.vector.tensor_tensor(scr[:], tp[:], hmask_sb[m][:],
                                            ALU.mult)
                    with nc.allow_low_precision("fp32 accum, fp32r round"):
                        nc.vector.tensor_reduce(
                            ctxT_sb[m][:, b:b + 1], scr[:],
                            mybir.AxisListType.X, ALU.add,
                        )

            # ---------------- prediction head ---------------------------
            hT_sb = []
            for hc in range(4):
                p = psv.tile([128, BPC], f32, name=f"hp{hc}", tag="vp")
                for m in range(4):
                    nc.tensor.matmul(
                        p[:], (w1T_sb[m][:, hc * 128:(hc + 1) * 128]),
                        (ctxT_sb[m][:]), start=(m == 0), stop=(m == 3),
                    )
                t1 = wp.tile([128, BPC], f32, name=f"t1_{hc}", tag="t1",
                             bufs=2)
                nc.vector.tensor_scalar(t1[:], p[:], b1_sb[:, hc:hc + 1],
                                        None, ALU.add)
                ht = cp.tile([128, BPC], f32r, name=f"hT{hc}", tag=f"hT{hc}")
                nc.vector.scalar_tensor_tensor(
                    ht[:], t1[:], 0.01, t1[:], ALU.mult, ALU.max
                )
                hT_sb.append(ht)
            r2p = pss.tile([NOUT, BPC], f32, name="r2p", tag="sp")
            for hc in range(4):
                nc.tensor.matmul(
                    r2p[:], (w2T_sb[hc][:]), (hT_sb[hc][:]),
                    start=(hc == 0), stop=(hc == 3),
                )
            r_sb = cp.tile([NOUT, BPC], f32r, name="r_sb", tag="r_sb")
            nc.vector.tensor_scalar(r_sb[:], r2p[:], b2_sb[:], 0.0,
                                    ALU.add, ALU.max)
            mp = pss.tile([1, BPC], f32, name="mp", tag="sp")
            nc.tensor.matmul(mp[:], (ones8_sb[:]), (r_sb[:]))
            mt = cp.tile([1, BPC], f32, name="mt", tag="mt")
            nc.vector.tensor_scalar(mt[:], mp[:], 1.0 / NOUT, None, ALU.mult)
            nc.vector.scalar_tensor_tensor(
                out_sb[:], mt[:], 0.01, mt[:], ALU.mult, ALU.max
            )
            nc.sync.dma_start(out=d_out[:], in_=out_sb[:])

    nc.compile()
    return nc


_CACHE = {}


def _get_module():
    if "nc" not in _CACHE:
        _CACHE["nc"] = _build()
    return _CACHE["nc"]


def _pos_encoding():
    pos = np.arange(L, dtype=np.float32)[:, None]
    div = np.exp(
        np.arange(0, D, 2, dtype=np.float32) * (-math.log(10000.0) / D)
    )
    pe = np.zeros((L, D), np.float32)
    pe[:, 0::2] = np.sin(pos * div)
    pe[:, 1::2] = np.cos(pos * div)
    return pe


def make_in_maps(data, lengths, emb, Wq, bq, Wk, bk, Wv, bv, W1, b1, W2, b2):
    # the kernel folds the K-projection into the score contraction; a
    # nonzero bk would add a per-head constant q.bk_h to the scores, which
    # this build omits (bk is zero for this module).
    assert float(np.abs(np.asarray(bk)).max()) == 0.0
    # V eviction is a plain copy; nonzero bv would need a bias add there.
    assert float(np.abs(np.asarray(bv)).max()) == 0.0

    pe = _pos_encoding()                       # [L, D]
    peT = np.zeros((E, LP), np.float32)
    peT[:, :L] = pe.T

    dpad = np.zeros((B, LP), np.int64)
    dpad[:, :L] = data
    data_f32 = dpad.astype(np.float32)

    p = (np.asarray(lengths).astype(np.int64) - 1)          # [B]
    idxl = np.asarray(data)[np.arange(B), p].astype(np.float32)
    pelT = pe[p].astype(np.float32).T                       # [D, B]

    bfl = ml_dtypes.bfloat16
    wb = np.concatenate(
        [np.asarray(Wq).T, np.asarray(Wk), np.asarray(Wv).T], axis=1
    ).astype(bfl)                                            # [512, 1536]
    wr = np.concatenate(
        [np.asarray(W1).T, np.asarray(W2).T], axis=1
    ).astype(np.float32)                                     # [512, 520]
    fB = np.concatenate(
        [np.asarray(bq).reshape(4, 128).T,
         np.asarray(b1).reshape(4, 128).T,
         np.arange(256, dtype=np.float32).reshape(2, 128).T], axis=1
    ).astype(np.float32)                                     # [128, 10]
    iota8 = np.broadcast_to(np.arange(LP, dtype=np.float32), (NH, LP))
    shared = {
        "emb": np.ascontiguousarray(emb, dtype=bfl),
        "wb": np.ascontiguousarray(wb),
        "wr": np.ascontiguousarray(wr),
        "peT": peT.astype(bfl),
        "id8b": np.eye(NH, dtype=bfl),
        "ones8": np.ones((NH, 1), np.float32),
        "fB": np.ascontiguousarray(fB),
    }
    in_maps = []
    for c in range(N_CORES):
        sl = slice(c * BPC, (c + 1) * BPC)
        m = dict(shared)
        m["drow"] = np.concatenate(
            [data_f32[sl].reshape(-1), idxl[sl]]
        ).reshape(1, -1).astype(bfl)
        m["fA"] = np.ascontiguousarray(np.concatenate(
            [pelT[:, sl], np.repeat(np.eye(NH, dtype=np.float32), DH, axis=0)],
            axis=1)).astype(np.float32)                      # [512, 12]
        m["fC"] = np.ascontiguousarray(np.concatenate(
            [np.broadcast_to(p[sl].astype(np.float32), (NH, BPC)),
             np.asarray(b2).reshape(NOUT, 1).astype(np.float32),
             iota8], axis=1)).astype(np.float32)             # [8, 1029]
        in_maps.append(m)
    return in_maps


def kernel(data, lengths, emb, Wq, bq, Wk, bk, Wv, bv, W1, b1, W2, b2):
    nc = _get_module()
    in_maps = make_in_maps(
        np.asarray(data), np.asarray(lengths), emb, Wq, bq, Wk, bk, Wv, bv,
        W1, b1, W2, b2,
    )
    res = run_bass_kernel_spmd(nc, in_maps, list(range(N_CORES)))
    out = np.concatenate(
        [res.results[c]["out"].reshape(BPC) for c in range(N_CORES)]
    )
    return out.astype(np.float32)


# revision 21
# speedup vs baseline: 1.0007x; 1.0007x over previous
"""Trainium2 Bass kernel for nn_Attention_module_52166672777937.

Data-parallel over batch across 8 NeuronCores (4 sequences per core).

Algorithmic restructuring (numerically validated against the reference):
the module only consumes the attention output at the LAST valid position
of each sequence (take_along_axis with lengths-1), and attention is
causal, so only ONE query row per sequence matters.  Consequences:

  * q is computed for a single position per sequence.
  * K is never materialized: scores = (qblk.T @ Wk) @ x.T, using
    associativity of the K projection with the score contraction.
  * softmax runs over [H=8, L] scores per sequence (no L x L matrix).
  * ctx = softmax(scores) @ V needs V = x @ Wv.T for all positions -- the
    dominant matmul, kept on TensorE at fp32r full rate.

Device layout: x is built in transposed [E, L] layout directly via a
one-hot matmul gather (onehot[c, l] = (data[l] == c), x.T = emb.T @
onehot + pe.T), which feeds both the score matmul and the V projection
without any transposes of large tensors.
"""

import math
import sys

import ml_dtypes
import numpy as np

sys.path.insert(0, "/opt/trn_rl_repo")

import concourse.bacc as bacc
import concourse.bass as bass
import concourse.mybir as mybir
import concourse.tile as tile
from concourse.bass_utils import run_bass_kernel_spmd

dt = mybir.dt
AF = mybir.ActivationFunctionType
ALU = mybir.AluOpType
PSUM = bass.MemorySpace.PSUM

N_CORES = 8
B, L = 32, 1000
LP = 1024                 # padded sequence length (2 x 512 column tiles)
TW = 512                  # column-tile width (max fp32 moving operand / PSUM bank)
NT = LP // TW             # column tiles per sequence
BPC = B // N_CORES        # sequences per core
NCH = 256                 # vocabulary
E = 512                   # embedding dim
D = 512                   # d_model
NH, DH = 8, 64            # heads
HS = 512                  # pred hidden size
NOUT = 8
NEG = -1.0e30
SCALE = 1.0 / math.sqrt(DH)


def _build():
    nc = bacc.Bacc(
        "TRN2", target_bir_lowering=False, debug=False, num_devices=N_CORES
    )

    f32 = dt.float32
    f32r = dt.float32r
    bf16 = dt.bfloat16
    # --- packed inputs (few wide DMAs instead of many narrow ones) ------
    # bf16s: data+idxlast row; emb; wqT|wk|wvT; peT; id8b
    d_drow = nc.dram_tensor("drow", [1, BPC * LP + BPC], bf16,
                            kind="ExternalInput")
    d_emb = nc.dram_tensor("emb", [NCH, E], bf16, kind="ExternalInput")
    d_wb = nc.dram_tensor("wb", [E, 3 * D], bf16, kind="ExternalInput")
    d_peT = nc.dram_tensor("peT", [E, LP], bf16, kind="ExternalInput")
    d_id8b = nc.dram_tensor("id8b", [NH, NH], bf16, kind="ExternalInput")
    # f32r: w1T|w2T ; ones8
    d_wr = nc.dram_tensor("wr", [D, HS + NOUT], f32r, kind="ExternalInput")
    d_ones8 = nc.dram_tensor("ones8", [NH, 1], f32r, kind="ExternalInput")
    # f32: pelastT|hmask (512-row); bq|b1|cvals (128-row); plast|b2|iota (8-row)
    d_fA = nc.dram_tensor("fA", [D, BPC + NH], f32, kind="ExternalInput")
    d_fB = nc.dram_tensor("fB", [128, 10], f32, kind="ExternalInput")
    d_fC = nc.dram_tensor("fC", [NH, BPC + 1 + LP], f32, kind="ExternalInput")
    d_out = nc.dram_tensor("out", [1, BPC], f32, kind="ExternalOutput")

    with tile.TileContext(nc) as tc:
        with (
            tc.tile_pool(name="const", bufs=1) as cp,
            tc.tile_pool(name="work", bufs=2) as wp,
            tc.tile_pool(name="psx", bufs=2, space=PSUM) as psx,
            tc.tile_pool(name="psv", bufs=2, space=PSUM) as psv,
            tc.tile_pool(name="pss", bufs=2, space=PSUM) as pss,
            tc.tile_pool(name="psc", bufs=2, space=PSUM) as psc,
        ):
            # ---------------- constant loads (packed) -------------------
            # critical path first: data row, broadcasts, emb, small consts
            dbb = []
            for b in range(BPC):
                t = cp.tile([128, LP], bf16, name=f"db{b}", tag=f"db{b}")
                nc.sync.dma_start(
                    out=t[:],
                    in_=d_drow[:, b * LP:(b + 1) * LP].to_broadcast(
                        (128, LP)),
                )
                dbb.append(t)
            drow_sb = cp.tile([1, BPC * LP + BPC], bf16, name="drow",
                              tag="drow")
            nc.sync.dma_start(out=drow_sb[:], in_=d_drow[:])
            data_row = drow_sb[:, 0:BPC * LP]
            idxb_sb = cp.tile([128, BPC], bf16, name="idxb", tag="idxb")
            nc.sync.dma_start(
                out=idxb_sb[:],
                in_=d_drow[:, BPC * LP:].to_broadcast((128, BPC)),
            )
            embp_sb = cp.tile([128, 2, E], bf16, name="embp", tag="embp")
            nc.sync.dma_start(
                out=embp_sb[:],
                in_=d_emb[:].rearrange("(c p) n -> p c n", p=128),
            )
            emb_sb = [embp_sb[:, c, :] for c in range(2)]
            fB_sb = cp.tile([128, 10], f32, name="fB", tag="fB")
            nc.sync.dma_start(out=fB_sb[:], in_=d_fB[:])
            bq_sb = fB_sb[:, 0:4]
            b1_sb = fB_sb[:, 4:8]
            cvals_sb = fB_sb[:, 8:10]
            fC_sb = cp.tile([NH, BPC + 1 + LP], f32, name="fC", tag="fC")
            nc.sync.dma_start(out=fC_sb[:], in_=d_fC[:])
            plast_sb = fC_sb[:, 0:BPC]
            b2_sb = fC_sb[:, BPC:BPC + 1]
            iota_sb = fC_sb[:, BPC + 1:]
            fA_sb = cp.tile([128, 4, BPC + NH], f32, name="fA", tag="fA")
            nc.sync.dma_start(
                out=fA_sb[:], in_=d_fA[:].rearrange("(c p) n -> p c n", p=128)
            )
            pelT_sb = [fA_sb[:, m, 0:BPC] for m in range(4)]
            hmask_sb = [fA_sb[:, m, BPC:BPC + NH] for m in range(4)]
            id8b_sb = cp.tile([NH, NH], bf16, name="id8b", tag="id8b")
            nc.sync.dma_start(out=id8b_sb[:], in_=d_id8b[:])
            ones8_sb = cp.tile([NH, 1], f32r, name="ones8", tag="ones8")
            nc.sync.dma_start(out=ones8_sb[:], in_=d_ones8[:])
            # bulk weights on the gpsimd queue (issues in parallel)
            wb_sb = cp.tile([128, 4, 3 * D], bf16, name="wb", tag="wb")
            nc.sync.dma_start(
                out=wb_sb[:], in_=d_wb[:].rearrange("(c p) n -> p c n", p=128)
            )
            wqT_sb = [wb_sb[:, e, 0:D] for e in range(4)]
            wk_sb = [wb_sb[:, c, D:2 * D] for c in range(4)]
            wvT_sb = [wb_sb[:, e, 2 * D:3 * D] for e in range(4)]
            peTp_sb = cp.tile([128, 4, LP], bf16, name="peTp", tag="peTp")
            nc.sync.dma_start(
                out=peTp_sb[:],
                in_=d_peT[:].rearrange("(c p) n -> p c n", p=128),
            )
            peT_sb = [peTp_sb[:, e, :] for e in range(4)]
            wr_sb = cp.tile([128, 4, HS + NOUT], f32r, name="wr", tag="wr")
            nc.sync.dma_start(
                out=wr_sb[:], in_=d_wr[:].rearrange("(c p) n -> p c n", p=128)
            )
            w1T_sb = [wr_sb[:, m, 0:HS] for m in range(4)]
            w2T_sb = [wr_sb[:, m, HS:HS + NOUT] for m in range(4)]

            madd_sb = [None] * BPC

            # ---------------- x_last gather -> q ------------------------
            ohl = []
            for c in range(2):
                t = cp.tile([128, BPC], bf16, name=f"ohl{c}", tag=f"ohl{c}")
                nc.vector.tensor_scalar(
                    t[:], idxb_sb[:], cvals_sb[:, c:c + 1], None, ALU.is_equal
                )
                ohl.append(t)
            # x_last.T [E, BPC] = emb.T @ onehot_last + pe_last.T
            xlast_sb = []
            for e in range(4):
                p = psx.tile([128, BPC], f32, name=f"xlp{e}", tag="xtp")
                for c in range(2):
                    nc.tensor.matmul(
                        p[:], (emb_sb[c][:, e * 128:(e + 1) * 128]),
                        (ohl[c][:]), start=(c == 0), stop=(c == 1),
                    )
                t = cp.tile([128, BPC], bf16, name=f"xlast{e}", tag=f"xlast{e}")
                nc.vector.tensor_tensor(t[:], p[:], pelT_sb[e][:], ALU.add)
                xlast_sb.append(t)
            # q.T [D, BPC] = Wq @ x_last.T + bq
            qT_sb = []
            for d in range(4):
                p = psv.tile([128, BPC], f32, name=f"qp{d}", tag="vp")
                for e in range(4):
                    nc.tensor.matmul(
                        p[:], (wqT_sb[e][:, d * 128:(d + 1) * 128]),
                        (xlast_sb[e][:]), start=(e == 0), stop=(e == 3),
                    )
                t = cp.tile([128, BPC], f32, name=f"qT{d}", tag=f"qT{d}")
                nc.vector.tensor_scalar(t[:], p[:], bq_sb[:, d:d + 1], None,
                                        ALU.add)
                qT_sb.append(t)

            # ---------------- main loop over sequences ------------------
            out_sb = cp.tile([1, BPC], f32, name="out_sb", tag="out_sb")
            ctxT_sb = [cp.tile([128, BPC], f32r, name=f"ctxT{m}", tag=f"ctxT{m}")
                       for m in range(4)]
            for b in range(BPC):
                # --- per-sequence qkvec = qblk.T @ Wk (K never formed) --
                qblk = []
                for d in range(4):
                    t = cp.tile([128, NH], bf16, name=f"qblk{b}_{d}",
                                tag=f"qblk{b}_{d}")
                    nc.vector.tensor_scalar(
                        t[:], hmask_sb[d][:], qT_sb[d][:, b:b + 1], None,
                        ALU.mult,
                    )
                    qblk.append(t)
                qkvp = pss.tile([NH, E], f32, name=f"qkvp{b}", tag="sp")
                for d in range(4):
                    nc.tensor.matmul(
                        qkvp[:], (qblk[d][:]), (wk_sb[d][:]),
                        start=(d == 0), stop=(d == 3),
                    )
                qkv_sb = wp.tile([NH, E], bf16, name=f"qkv{b}", tag="qkv",
                                 bufs=2)
                nc.vector.tensor_copy(qkv_sb[:], qkvp[:])
                qkvT = []
                for e in range(4):
                    tp = pss.tile([128, NH], bf16, name=f"qkvTp{b}_{e}",
                                  tag="sp")
                    nc.tensor.transpose(
                        tp[:], qkv_sb[:, e * 128:(e + 1) * 128], id8b_sb[:]
                    )
                    t = cp.tile([128, NH], bf16, name=f"qkvT{b}_{e}",
                                tag=f"qkvT{b}_{e}")
                    nc.vector.tensor_copy(t[:], tp[:])
                    qkvT.append(t)

                # --- attention over the sequence ------------------------
                # causal/validity mask, additive (0 valid / -1e30 invalid);
                # generated just-in-time so it doesn't block the first tile
                m8 = cp.tile([NH, LP], f32, name=f"madd{b}", tag=f"madd{b}")
                nc.vector.tensor_scalar(
                    m8[:], iota_sb[:], plast_sb[:, b:b + 1], NEG,
                    ALU.is_gt, ALU.mult,
                )
                madd_sb[b] = m8
                ctxp = psc.tile([NH, D], f32, name=f"ctx{b}", tag="cp")
                den_sb = wp.tile([NH, NT], f32, name=f"den{b}", tag="den",
                                 bufs=2)
                for t in range(NT):
                    col0 = b * LP + t * TW
                    # one-hot [256, TW] for this tile's positions
                    oh = []
                    for c in range(2):
                        o = wp.tile([128, TW], bf16, name=f"oh{b}_{t}_{c}",
                                    tag="oh", bufs=6)
                        nc.vector.tensor_scalar(
                            o[:], dbb[b][:, t * TW:(t + 1) * TW],
                            cvals_sb[:, c:c + 1], None, ALU.is_equal,
                        )
                        oh.append(o)
                    # x.T tile [E, TW] = emb.T @ onehot + pe.T
                    xT = []
                    for e in range(4):
                        p = psx.tile([128, TW], f32, name=f"xtp{b}_{t}_{e}",
                                     tag="xtp")
                        for c in range(2):
                            nc.tensor.matmul(
                                p[:],
                                (emb_sb[c][:, e * 128:(e + 1) * 128]),
                                (oh[c][:]), start=(c == 0), stop=(c == 1),
                            )
                        x = wp.tile([128, TW], bf16, name=f"xT{b}_{t}_{e}",
                                    tag=f"xT{e}", bufs=3)
                        nc.vector.tensor_tensor(
                            x[:], p[:], peT_sb[e][:, t * TW:(t + 1) * TW],
                            ALU.add,
                        )
                        xT.append(x)
                    # scores [NH, TW] = qkvec @ x.T
                    sp = pss.tile([NH, TW], f32, name=f"s{b}_{t}", tag="sp")
                    for e in range(4):
                        nc.tensor.matmul(
                            sp[:], (qkvT[e][:]), (xT[e][:]),
                            start=(e == 0), stop=(e == 3),
                        )
                    # masked scaled scores -> exp (+ row-sum for denom)
                    sm = wp.tile([NH, TW], f32, name=f"sm{b}_{t}", tag="sm",
                                 bufs=3)
                    nc.vector.scalar_tensor_tensor(
                        sm[:], sp[:], SCALE, madd_sb[b][:, t * TW:(t + 1) * TW],
                        ALU.mult, ALU.add,
                    )
                    ex = wp.tile([NH, TW], bf16, name=f"ex{b}_{t}", tag="ex",
                                 bufs=3)
                    nc.scalar.activation(
                        ex[:], sm[:], AF.Exp,
                        accum_out=den_sb[:, t:t + 1],
                    )
                    # V tile + ctx accumulation, one 128-row block at a time
                    for lc in range(4):
                        ap = pss.tile([128, NH], bf16, name=f"aTp{b}_{t}_{lc}",
                                      tag="sp")
                        nc.tensor.transpose(
                            ap[:], ex[:, lc * 128:(lc + 1) * 128], id8b_sb[:]
                        )
                        aT = wp.tile([128, NH], bf16, name=f"aT{b}_{t}_{lc}",
                                     tag="aT", bufs=8)
                        nc.scalar.copy(aT[:], ap[:])
                        vp = psv.tile([128, D], f32, name=f"vp{b}_{t}_{lc}",
                                      tag="vp")
                        for e in range(4):
                            nc.tensor.matmul(
                                vp[:],
                                (xT[e][:, lc * 128:(lc + 1) * 128]),
                                (wvT_sb[e][:]),
                                start=(e == 0), stop=(e == 3),
                            )
                        v = wp.tile([128, D], bf16, name=f"v{b}_{t}_{lc}",
                                    tag="v", bufs=4)
                        if lc % 2 == 0:
                            nc.vector.tensor_copy(v[:], vp[:])
                        else:
                            nc.scalar.copy(v[:], vp[:])
                        nc.tensor.matmul(
                            ctxp[:], (aT[:]), (v[:]),
                            start=(t == 0 and lc == 0),
                            stop=(t == NT - 1 and lc == 3),
                        )
                # normalize ctx rows by the masked softmax denominator
                dsum = wp.tile([NH, 1], f32, name=f"dsum{b}", tag="dsum",
                               bufs=2)
                nc.vector.reduce_sum(dsum[:], den_sb[:],
                                     axis=mybir.AxisListType.X)
                rec = wp.tile([NH, 1], f32, name=f"rec{b}", tag="rec", bufs=2)
                nc.vector.reciprocal(rec[:], dsum[:])
                ctx_sb = wp.tile([NH, D], bf16, name=f"ctxs{b}", tag="ctxs",
                                 bufs=2)
                nc.scalar.activation(ctx_sb[:], ctxp[:], AF.Copy,
                                     scale=rec[:])
                # extract block-diagonal -> ctx.T [D, BPC] column b
                for m in range(4):
                    tp = pss.tile([128, NH], bf16, name=f"ctp{b}_{m}", tag="sp")
                    nc.tensor.transpose(
                        tp[:], ctx_sb[:, m * 128:(m + 1) * 128], id8b_sb[:]
                    )
                    scr = wp.tile([128, NH], f32, name=f"scr{b}_{m}",
                                  tag="scr", bufs=2)
                    nc.vector.tensor_tensor(scr[:], tp[:], hmask_sb[m][:],
                                            ALU.mult)
                    with nc.allow_low_precision("fp32 accum, fp32r round"):
                        nc.vector.tensor_reduce(
                            ctxT_sb[m][:, b:b + 1], scr[:],
                            mybir.AxisListType.X, ALU.add,
                        )

            # ---------------- prediction head ---------------------------
            hT_sb = []
            for hc in range(4):
                p = psv.tile([128, BPC], f32, name=f"hp{hc}", tag="vp")
                for m in range(4):
                    nc.tensor.matmul(
                        p[:], (w1T_sb[m][:, hc * 128:(hc + 1) * 128]),
                        (ctxT_sb[m][:]), start=(m == 0), stop=(m == 3),
                    )
                t1 = wp.tile([128, BPC], f32, name=f"t1_{hc}", tag="t1",
                             bufs=2)
                nc.vector.tensor_scalar(t1[:], p[:], b1_sb[:, hc:hc + 1],
                                        None, ALU.add)
                ht = cp.tile([128, BPC], f32r, name=f"hT{hc}", tag=f"hT{hc}")
                nc.vector.scalar_tensor_tensor(
                    ht[:], t1[:], 0.01, t1[:], ALU.mult, ALU.max
                )
                hT_sb.append(ht)
            r2p = pss.tile([NOUT, BPC], f32, name="r2p", tag="sp")
            for hc in range(4):
                nc.tensor.matmul(
                    r2p[:], (w2T_sb[hc][:]), (hT_sb[hc][:]),
                    start=(hc == 0), stop=(hc == 3),
                )
            r_sb = cp.tile([NOUT, BPC], f32r, name="r_sb", tag="r_sb")
            nc.vector.tensor_scalar(r_sb[:], r2p[:], b2_sb[:], 0.0,
                                    ALU.add, ALU.max)
            mp = pss.tile([1, BPC], f32, name="mp", tag="sp")
            nc.tensor.matmul(mp[:], (ones8_sb[:]), (r_sb[:]))
            mt = cp.tile([1, BPC], f32, name="mt", tag="mt")
            nc.vector.tensor_scalar(mt[:], mp[:], 1.0 / NOUT, None, ALU.mult)
            nc.vector.scalar_tensor_tensor(
                out_sb[:], mt[:], 0.01, mt[:], ALU.mult, ALU.max
            )
            nc.sync.dma_start(out=d_out[:], in_=out_sb[:])

    nc.compile()
    return nc


_CACHE = {}


def _get_module():
    if "nc" not in _CACHE:
        _CACHE["nc"] = _build()
    return _CACHE["nc"]


def _pos_encoding():
    pos = np.arange(L, dtype=np.float32)[:, None]
    div = np.exp(
        np.arange(0, D, 2, dtype=np.float32) * (-math.log(10000.0) / D)
    )
    pe = np.zeros((L, D), np.float32)
    pe[:, 0::2] = np.sin(pos * div)
    pe[:, 1::2] = np.cos(pos * div)
    return pe


def make_in_maps(data, lengths, emb, Wq, bq, Wk, bk, Wv, bv, W1, b1, W2, b2):
    # the kernel folds the K-projection into the score contraction; a
    # nonzero bk would add a per-head constant q.bk_h to the scores, which
    # this build omits (bk is zero for this module).
    assert float(np.abs(np.asarray(bk)).max()) == 0.0
    # V eviction is a plain copy; nonzero bv would need a bias add there.
    assert float(np.abs(np.asarray(bv)).max()) == 0.0

    pe = _pos_encoding()                       # [L, D]
    peT = np.zeros((E, LP), np.float32)
    peT[:, :L] = pe.T

    dpad = np.zeros((B, LP), np.int64)
    dpad[:, :L] = data
    data_f32 = dpad.astype(np.float32)

    p = (np.asarray(lengths).astype(np.int64) - 1)          # [B]
    idxl = np.asarray(data)[np.arange(B), p].astype(np.float32)
    pelT = pe[p].astype(np.float32).T                       # [D, B]

    bfl = ml_dtypes.bfloat16
    wb = np.concatenate(
        [np.asarray(Wq).T, np.asarray(Wk), np.asarray(Wv).T], axis=1
    ).astype(bfl)                                            # [512, 1536]
    wr = np.concatenate(
        [np.asarray(W1).T, np.asarray(W2).T], axis=1
    ).astype(np.float32)                                     # [512, 520]
    fB = np.concatenate(
        [np.asarray(bq).reshape(4, 128).T,
         np.asarray(b1).reshape(4, 128).T,
         np.arange(256, dtype=np.float32).reshape(2, 128).T], axis=1
    ).astype(np.float32)                                     # [128, 10]
    iota8 = np.broadcast_to(np.arange(LP, dtype=np.float32), (NH, LP))
    shared = {
        "emb": np.ascontiguousarray(emb, dtype=bfl),
        "wb": np.ascontiguousarray(wb),
        "wr": np.ascontiguousarray(wr),
        "peT": peT.astype(bfl),
        "id8b": np.eye(NH, dtype=bfl),
        "ones8": np.ones((NH, 1), np.float32),
        "fB": np.ascontiguousarray(fB),
    }
    in_maps = []
    for c in range(N_CORES):
        sl = slice(c * BPC, (c + 1) * BPC)
        m = dict(shared)
        m["drow"] = np.concatenate(
            [data_f32[sl].reshape(-1), idxl[sl]]
        ).reshape(1, -1).astype(bfl)
        m["fA"] = np.ascontiguousarray(np.concatenate(
            [pelT[:, sl], np.repeat(np.eye(NH, dtype=np.float32), DH, axis=0)],
            axis=1)).astype(np.float32)                      # [512, 12]
        m["fC"] = np.ascontiguousarray(np.concatenate(
            [np.broadcast_to(p[sl].astype(np.float32), (NH, BPC)),
             np.asarray(b2).reshape(NOUT, 1).astype(np.float32),
             iota8], axis=1)).astype(np.float32)             # [8, 1029]
        in_maps.append(m)
    return in_maps


def kernel(data, lengths, emb, Wq, bq, Wk, bk, Wv, bv, W1, b1, W2, b2):
    nc = _get_module()
    in_maps = make_in_maps(
        np.asarray(data), np.asarray(lengths), emb, Wq, bq, Wk, bk, Wv, bv,
        W1, b1, W2, b2,
    )
    res = run_bass_kernel_spmd(nc, in_maps, list(range(N_CORES)))
    out = np.concatenate(
        [res.results[c]["out"].reshape(BPC) for c in range(N_CORES)]
    )
    return out.astype(np.float32)


# revision 22
# speedup vs baseline: 1.0298x; 1.0291x over previous
"""Trainium2 Bass kernel for nn_Attention_module_52166672777937.

Data-parallel over batch across 8 NeuronCores (4 sequences per core).

Algorithmic restructuring (numerically validated against the reference):
the module only consumes the attention output at the LAST valid position
of each sequence (take_along_axis with lengths-1), and attention is
causal, so only ONE query row per sequence matters.  Consequences:

  * q is computed for a single position per sequence.
  * K is never materialized: scores = (qblk.T @ Wk) @ x.T, using
    associativity of the K projection with the score contraction.
  * softmax runs over [H=8, L] scores per sequence (no L x L matrix).
  * ctx = softmax(scores) @ V needs V = x @ Wv.T for all positions -- the
    dominant matmul, kept on TensorE at fp32r full rate.

Device layout: x is built in transposed [E, L] layout directly via a
one-hot matmul gather (onehot[c, l] = (data[l] == c), x.T = emb.T @
onehot + pe.T), which feeds both the score matmul and the V projection
without any transposes of large tensors.
"""

import math
import sys

import ml_dtypes
import numpy as np

sys.path.insert(0, "/opt/trn_rl_repo")

import concourse.bacc as bacc
import concourse.bass as bass
import concourse.mybir as mybir
import concourse.tile as tile
from concourse.bass_utils import run_bass_kernel_spmd

dt = mybir.dt
AF = mybir.ActivationFunctionType
ALU = mybir.AluOpType
PSUM = bass.MemorySpace.PSUM

N_CORES = 8
B, L = 32, 1000
LP = 1024                 # padded sequence length (2 x 512 column tiles)
TW = 512                  # column-tile width (max fp32 moving operand / PSUM bank)
NT = LP // TW             # column tiles per sequence
BPC = B // N_CORES        # sequences per core
NCH = 256                 # vocabulary
E = 512                   # embedding dim
D = 512                   # d_model
NH, DH = 8, 64            # heads
HS = 512                  # pred hidden size
NOUT = 8
NEG = -1.0e30
SCALE = 1.0 / math.sqrt(DH)


def _build():
    nc = bacc.Bacc(
        "TRN2", target_bir_lowering=False, debug=False, num_devices=N_CORES
    )

    f32 = dt.float32
    f32r = dt.float32r
    bf16 = dt.bfloat16
    # --- packed inputs (few wide DMAs instead of many narrow ones) ------
    # bf16s: data+idxlast row; emb; wqT|wk|wvT; peT; id8b
    d_drow = nc.dram_tensor("drow", [1, BPC * LP + BPC], bf16,
                            kind="ExternalInput")
    d_emb = nc.dram_tensor("emb", [NCH, E], bf16, kind="ExternalInput")
    d_wb = nc.dram_tensor("wb", [E, 3 * D], bf16, kind="ExternalInput")
    d_peT = nc.dram_tensor("peT", [E, LP], bf16, kind="ExternalInput")
    d_id8b = nc.dram_tensor("id8b", [NH, NH], bf16, kind="ExternalInput")
    # f32r: w1T|w2T ; ones8
    d_wr = nc.dram_tensor("wr", [D, HS + NOUT], f32r, kind="ExternalInput")
    d_ones8 = nc.dram_tensor("ones8", [NH, 1], f32r, kind="ExternalInput")
    # f32: pelastT|hmask (512-row); bq|b1|cvals (128-row); plast|b2|iota (8-row)
    d_fA = nc.dram_tensor("fA", [D, BPC + NH], f32, kind="ExternalInput")
    d_fB = nc.dram_tensor("fB", [128, 10], f32, kind="ExternalInput")
    d_fC = nc.dram_tensor("fC", [NH, BPC + 1 + LP], f32, kind="ExternalInput")
    d_out = nc.dram_tensor("out", [1, BPC], f32, kind="ExternalOutput")

    with tile.TileContext(nc) as tc:
        with (
            tc.tile_pool(name="const", bufs=1) as cp,
            tc.tile_pool(name="work", bufs=2) as wp,
            tc.tile_pool(name="psx", bufs=2, space=PSUM) as psx,
            tc.tile_pool(name="psv", bufs=2, space=PSUM) as psv,
            tc.tile_pool(name="pss", bufs=2, space=PSUM) as pss,
            tc.tile_pool(name="psc", bufs=2, space=PSUM) as psc,
        ):
            # ---------------- constant loads (packed) -------------------
            # issued in order of first use so compute starts early:
            # fB(cvals) -> db0 -> emb -> wq -> wk -> peT -> wv -> ...
            fB_sb = cp.tile([128, 10], f32, name="fB", tag="fB")
            nc.sync.dma_start(out=fB_sb[:], in_=d_fB[:])
            bq_sb = fB_sb[:, 0:4]
            b1_sb = fB_sb[:, 4:8]
            cvals_sb = fB_sb[:, 8:10]
            dbb = []
            for b in range(BPC):
                t = cp.tile([128, LP], bf16, name=f"db{b}", tag=f"db{b}")
                dbb.append(t)
            nc.sync.dma_start(
                out=dbb[0][:],
                in_=d_drow[:, 0:LP].to_broadcast((128, LP)),
            )
            embp_sb = cp.tile([128, 2, E], bf16, name="embp", tag="embp")
            nc.sync.dma_start(
                out=embp_sb[:],
                in_=d_emb[:].rearrange("(c p) n -> p c n", p=128),
            )
            emb_sb = [embp_sb[:, c, :] for c in range(2)]
            wqp_sb = cp.tile([128, 4, D], bf16, name="wqp", tag="wqp")
            nc.sync.dma_start(
                out=wqp_sb[:],
                in_=d_wb[:, 0:D].rearrange("(c p) n -> p c n", p=128),
            )
            wqT_sb = [wqp_sb[:, e, :] for e in range(4)]
            wkp_sb = cp.tile([128, 4, D], bf16, name="wkp", tag="wkp")
            nc.sync.dma_start(
                out=wkp_sb[:],
                in_=d_wb[:, D:2 * D].rearrange("(c p) n -> p c n", p=128),
            )
            wk_sb = [wkp_sb[:, c, :] for c in range(4)]
            peTp_sb = cp.tile([128, 4, LP], bf16, name="peTp", tag="peTp")
            nc.sync.dma_start(
                out=peTp_sb[:],
                in_=d_peT[:].rearrange("(c p) n -> p c n", p=128),
            )
            peT_sb = [peTp_sb[:, e, :] for e in range(4)]
            wvp_sb = cp.tile([128, 4, D], bf16, name="wvp", tag="wvp")
            nc.sync.dma_start(
                out=wvp_sb[:],
                in_=d_wb[:, 2 * D:3 * D].rearrange("(c p) n -> p c n", p=128),
            )
            wvT_sb = [wvp_sb[:, e, :] for e in range(4)]
            idxb_sb = cp.tile([128, BPC], bf16, name="idxb", tag="idxb")
            nc.sync.dma_start(
                out=idxb_sb[:],
                in_=d_drow[:, BPC * LP:].to_broadcast((128, BPC)),
            )
            fA_sb = cp.tile([128, 4, BPC + NH], f32, name="fA", tag="fA")
            nc.sync.dma_start(
                out=fA_sb[:], in_=d_fA[:].rearrange("(c p) n -> p c n", p=128)
            )
            pelT_sb = [fA_sb[:, m, 0:BPC] for m in range(4)]
            hmask_sb = [fA_sb[:, m, BPC:BPC + NH] for m in range(4)]
            fC_sb = cp.tile([NH, BPC + 1 + LP], f32, name="fC", tag="fC")
            nc.sync.dma_start(out=fC_sb[:], in_=d_fC[:])
            plast_sb = fC_sb[:, 0:BPC]
            b2_sb = fC_sb[:, BPC:BPC + 1]
            iota_sb = fC_sb[:, BPC + 1:]
            id8b_sb = cp.tile([NH, NH], bf16, name="id8b", tag="id8b")
            nc.sync.dma_start(out=id8b_sb[:], in_=d_id8b[:])
            for b in range(1, BPC):
                nc.sync.dma_start(
                    out=dbb[b][:],
                    in_=d_drow[:, b * LP:(b + 1) * LP].to_broadcast(
                        (128, LP)),
                )
            drow_sb = cp.tile([1, BPC * LP + BPC], bf16, name="drow",
                              tag="drow")
            nc.sync.dma_start(out=drow_sb[:], in_=d_drow[:])
            wr_sb = cp.tile([128, 4, HS + NOUT], f32r, name="wr", tag="wr")
            nc.sync.dma_start(
                out=wr_sb[:], in_=d_wr[:].rearrange("(c p) n -> p c n", p=128)
            )
            w1T_sb = [wr_sb[:, m, 0:HS] for m in range(4)]
            w2T_sb = [wr_sb[:, m, HS:HS + NOUT] for m in range(4)]
            ones8_sb = cp.tile([NH, 1], f32r, name="ones8", tag="ones8")
            nc.sync.dma_start(out=ones8_sb[:], in_=d_ones8[:])

            madd_sb = [None] * BPC

            # ---------------- x_last gather -> q ------------------------
            ohl = []
            for c in range(2):
                t = cp.tile([128, BPC], bf16, name=f"ohl{c}", tag=f"ohl{c}")
                nc.vector.tensor_scalar(
                    t[:], idxb_sb[:], cvals_sb[:, c:c + 1], None, ALU.is_equal
                )
                ohl.append(t)
            # x_last.T [E, BPC] = emb.T @ onehot_last + pe_last.T
            xlast_sb = []
            for e in range(4):
                p = psx.tile([128, BPC], f32, name=f"xlp{e}", tag="xtp")
                for c in range(2):
                    nc.tensor.matmul(
                        p[:], (emb_sb[c][:, e * 128:(e + 1) * 128]),
                        (ohl[c][:]), start=(c == 0), stop=(c == 1),
                    )
                t = cp.tile([128, BPC], bf16, name=f"xlast{e}", tag=f"xlast{e}")
                nc.vector.tensor_tensor(t[:], p[:], pelT_sb[e][:], ALU.add)
                xlast_sb.append(t)
            # q.T [D, BPC] = Wq @ x_last.T + bq
            qT_sb = []
            for d in range(4):
                p = psv.tile([128, BPC], f32, name=f"qp{d}", tag="vp")
                for e in range(4):
                    nc.tensor.matmul(
                        p[:], (wqT_sb[e][:, d * 128:(d + 1) * 128]),
                        (xlast_sb[e][:]), start=(e == 0), stop=(e == 3),
                    )
                t = cp.tile([128, BPC], f32, name=f"qT{d}", tag=f"qT{d}")
                nc.vector.tensor_scalar(t[:], p[:], bq_sb[:, d:d + 1], None,
                                        ALU.add)
                qT_sb.append(t)

            # ---------------- main loop over sequences ------------------
            out_sb = cp.tile([1, BPC], f32, name="out_sb", tag="out_sb")
            ctxT_sb = [cp.tile([128, BPC], f32r, name=f"ctxT{m}", tag=f"ctxT{m}")
                       for m in range(4)]
            for b in range(BPC):
                # --- per-sequence qkvec = qblk.T @ Wk (K never formed) --
                qblk = []
                for d in range(4):
                    t = cp.tile([128, NH], bf16, name=f"qblk{b}_{d}",
                                tag=f"qblk{b}_{d}")
                    nc.vector.tensor_scalar(
                        t[:], hmask_sb[d][:], qT_sb[d][:, b:b + 1], None,
                        ALU.mult,
                    )
                    qblk.append(t)
                qkvp = pss.tile([NH, E], f32, name=f"qkvp{b}", tag="sp")
                for d in range(4):
                    nc.tensor.matmul(
                        qkvp[:], (qblk[d][:]), (wk_sb[d][:]),
                        start=(d == 0), stop=(d == 3),
                    )
                qkv_sb = wp.tile([NH, E], bf16, name=f"qkv{b}", tag="qkv",
                                 bufs=2)
                nc.vector.tensor_copy(qkv_sb[:], qkvp[:])
                qkvT = []
                for e in range(4):
                    tp = pss.tile([128, NH], bf16, name=f"qkvTp{b}_{e}",
                                  tag="sp")
                    nc.tensor.transpose(
                        tp[:], qkv_sb[:, e * 128:(e + 1) * 128], id8b_sb[:]
                    )
                    t = cp.tile([128, NH], bf16, name=f"qkvT{b}_{e}",
                                tag=f"qkvT{b}_{e}")
                    nc.vector.tensor_copy(t[:], tp[:])
                    qkvT.append(t)

                # --- attention over the sequence ------------------------
                # causal/validity mask, additive (0 valid / -1e30 invalid);
                # generated just-in-time so it doesn't block the first tile
                m8 = cp.tile([NH, LP], f32, name=f"madd{b}", tag=f"madd{b}")
                nc.vector.tensor_scalar(
                    m8[:], iota_sb[:], plast_sb[:, b:b + 1], NEG,
                    ALU.is_gt, ALU.mult,
                )
                madd_sb[b] = m8
                ctxp = psc.tile([NH, D], f32, name=f"ctx{b}", tag="cp")
                den_sb = wp.tile([NH, NT], f32, name=f"den{b}", tag="den",
                                 bufs=2)
                for t in range(NT):
                    col0 = b * LP + t * TW
                    # one-hot [256, TW] for this tile's positions
                    oh = []
                    for c in range(2):
                        o = wp.tile([128, TW], bf16, name=f"oh{b}_{t}_{c}",
                                    tag="oh", bufs=6)
                        nc.vector.tensor_scalar(
                            o[:], dbb[b][:, t * TW:(t + 1) * TW],
                            cvals_sb[:, c:c + 1], None, ALU.is_equal,
                        )
                        oh.append(o)
                    # x.T tile [E, TW] = emb.T @ onehot + pe.T
                    xT = []
                    for e in range(4):
                        p = psx.tile([128, TW], f32, name=f"xtp{b}_{t}_{e}",
                                     tag="xtp")
                        for c in range(2):
                            nc.tensor.matmul(
                                p[:],
                                (emb_sb[c][:, e * 128:(e + 1) * 128]),
                                (oh[c][:]), start=(c == 0), stop=(c == 1),
                            )
                        x = wp.tile([128, TW], bf16, name=f"xT{b}_{t}_{e}",
                                    tag=f"xT{e}", bufs=3)
                        nc.vector.tensor_tensor(
                            x[:], p[:], peT_sb[e][:, t * TW:(t + 1) * TW],
                            ALU.add,
                        )
                        xT.append(x)
                    # scores [NH, TW] = qkvec @ x.T
                    sp = pss.tile([NH, TW], f32, name=f"s{b}_{t}", tag="sp")
                    for e in range(4):
                        nc.tensor.matmul(
                            sp[:], (qkvT[e][:]), (xT[e][:]),
                            start=(e == 0), stop=(e == 3),
                        )
                    # masked scaled scores -> exp (+ row-sum for denom)
                    sm = wp.tile([NH, TW], f32, name=f"sm{b}_{t}", tag="sm",
                                 bufs=3)
                    nc.vector.scalar_tensor_tensor(
                        sm[:], sp[:], SCALE, madd_sb[b][:, t * TW:(t + 1) * TW],
                        ALU.mult, ALU.add,
                    )
                    ex = wp.tile([NH, TW], bf16, name=f"ex{b}_{t}", tag="ex",
                                 bufs=3)
                    nc.scalar.activation(
                        ex[:], sm[:], AF.Exp,
                        accum_out=den_sb[:, t:t + 1],
                    )
                    # V tile + ctx accumulation, one 128-row block at a time
                    for lc in range(4):
                        ap = pss.tile([128, NH], bf16, name=f"aTp{b}_{t}_{lc}",
                                      tag="sp")
                        nc.tensor.transpose(
                            ap[:], ex[:, lc * 128:(lc + 1) * 128], id8b_sb[:]
                        )
                        aT = wp.tile([128, NH], bf16, name=f"aT{b}_{t}_{lc}",
                                     tag="aT", bufs=8)
                        nc.scalar.copy(aT[:], ap[:])
                        vp = psv.tile([128, D], f32, name=f"vp{b}_{t}_{lc}",
                                      tag="vp")
                        for e in range(4):
                            nc.tensor.matmul(
                                vp[:],
                                (xT[e][:, lc * 128:(lc + 1) * 128]),
                                (wvT_sb[e][:]),
                                start=(e == 0), stop=(e == 3),
                            )
                        v = wp.tile([128, D], bf16, name=f"v{b}_{t}_{lc}",
                                    tag="v", bufs=4)
                        if lc % 2 == 0:
                            nc.vector.tensor_copy(v[:], vp[:])
                        else:
                            nc.scalar.copy(v[:], vp[:])
                        nc.tensor.matmul(
                            ctxp[:], (aT[:]), (v[:]),
                            start=(t == 0 and lc == 0),
                            stop=(t == NT - 1 and lc == 3),
                        )
                # normalize ctx rows by the masked softmax denominator
                dsum = wp.tile([NH, 1], f32, name=f"dsum{b}", tag="dsum",
                               bufs=2)
                nc.vector.reduce_sum(dsum[:], den_sb[:],
                                     axis=mybir.AxisListType.X)
                rec = wp.tile([NH, 1], f32, name=f"rec{b}", tag="rec", bufs=2)
                nc.vector.reciprocal(rec[:], dsum[:])
                ctx_sb = wp.tile([NH, D], bf16, name=f"ctxs{b}", tag="ctxs",
                                 bufs=2)
                nc.scalar.activation(ctx_sb[:], ctxp[:], AF.Copy,
                                     scale=rec[:])
                # extract block-diagonal -> ctx.T [D, BPC] column b
                for m in range(4):
                    tp = pss.tile([128, NH], bf16, name=f"ctp{b}_{m}", tag="sp")
                    nc.tensor.transpose(
                        tp[:], ctx_sb[:, m * 128:(m + 1) * 128], id8b_sb[:]
                    )
                    scr = wp.tile([128, NH], f32, name=f"scr{b}_{m}",
                                  tag="scr", bufs=2)
                    nc.vector.tensor_tensor(scr[:], tp[:], hmask_sb[m][:],
                                            ALU.mult)
                    with nc.allow_low_precision("fp32 accum, fp32r round"):
                        nc.vector.tensor_reduce(
                            ctxT_sb[m][:, b:b + 1], scr[:],
                            mybir.AxisListType.X, ALU.add,
                        )

            # ---------------- prediction head ---------------------------
            hT_sb = []
            for hc in range(4):
                p = psv.tile([128, BPC], f32, name=f"hp{hc}", tag="vp")
                for m in range(4):
                    nc.tensor.matmul(
                        p[:], (w1T_sb[m][:, hc * 128:(hc + 1) * 128]),
                        (ctxT_sb[m][:]), start=(m == 0), stop=(m == 3),
                    )
                t1 = wp.tile([128, BPC], f32, name=f"t1_{hc}", tag="t1",
                             bufs=2)
                nc.vector.tensor_scalar(t1[:], p[:], b1_sb[:, hc:hc + 1],
                                        None, ALU.add)
                ht = cp.tile([128, BPC], f32r, name=f"hT{hc}", tag=f"hT{hc}")
                nc.vector.scalar_tensor_tensor(
                    ht[:], t1[:], 0.01, t1[:], ALU.mult, ALU.max
                )
                hT_sb.append(ht)
            r2p = pss.tile([NOUT, BPC], f32, name="r2p", tag="sp")
            for hc in range(4):
                nc.tensor.matmul(
                    r2p[:], (w2T_sb[hc][:]), (hT_sb[hc][:]),
                    start=(hc == 0), stop=(hc == 3),
                )
            r_sb = cp.tile([NOUT, BPC], f32r, name="r_sb", tag="r_sb")
            nc.vector.tensor_scalar(r_sb[:], r2p[:], b2_sb[:], 0.0,
                                    ALU.add, ALU.max)
            mp = pss.tile([1, BPC], f32, name="mp", tag="sp")
            nc.tensor.matmul(mp[:], (ones8_sb[:]), (r_sb[:]))
            mt = cp.tile([1, BPC], f32, name="mt", tag="mt")
            nc.vector.tensor_scalar(mt[:], mp[:], 1.0 / NOUT, None, ALU.mult)
            nc.vector.scalar_tensor_tensor(
                out_sb[:], mt[:], 0.01, mt[:], ALU.mult, ALU.max
            )
            nc.sync.dma_start(out=d_out[:], in_=out_sb[:])

    nc.compile()
    return nc


_CACHE = {}


def _get_module():
    if "nc" not in _CACHE:
        _CACHE["nc"] = _build()
    return _CACHE["nc"]


def _pos_encoding():
    pos = np.arange(L, dtype=np.float32)[:, None]
    div = np.exp(
        np.arange(0, D, 2, dtype=np.float32) * (-math.log(10000.0) / D)
    )
    pe = np.zeros((L, D), np.float32)
    pe[:, 0::2] = np.sin(pos * div)
    pe[:, 1::2] = np.cos(pos * div)
    return pe


def make_in_maps(data, lengths, emb, Wq, bq, Wk, bk, Wv, bv, W1, b1, W2, b2):
    # the kernel folds the K-projection into the score contraction; a
    # nonzero bk would add a per-head constant q.bk_h to the scores, which
    # this build omits (bk is zero for this module).
    assert float(np.abs(np.asarray(bk)).max()) == 0.0
    # V eviction is a plain copy; nonzero bv would need a bias add there.
    assert float(np.abs(np.asarray(bv)).max()) == 0.0

    pe = _pos_encoding()                       # [L, D]
    peT = np.zeros((E, LP), np.float32)
    peT[:, :L] = pe.T

    dpad = np.zeros((B, LP), np.int64)
    dpad[:, :L] = data
    data_f32 = dpad.astype(np.float32)

    p = (np.asarray(lengths).astype(np.int64) - 1)          # [B]
    idxl = np.asarray(data)[np.arange(B), p].astype(np.float32)
    pelT = pe[p].astype(np.float32).T                       # [D, B]

    bfl = ml_dtypes.bfloat16
    wb = np.concatenate(
        [np.asarray(Wq).T, np.asarray(Wk), np.asarray(Wv).T], axis=1
    ).astype(bfl)                                            # [512, 1536]
    wr = np.concatenate(
        [np.asarray(W1).T, np.asarray(W2).T], axis=1
    ).astype(np.float32)                                     # [512, 520]
    fB = np.concatenate(
        [np.asarray(bq).reshape(4, 128).T,
         np.asarray(b1).reshape(4, 128).T,
         np.arange(256, dtype=np.float32).reshape(2, 128).T], axis=1
    ).astype(np.float32)                                     # [128, 10]
    iota8 = np.broadcast_to(np.arange(LP, dtype=np.float32), (NH, LP))
    shared = {
        "emb": np.ascontiguousarray(emb, dtype=bfl),
        "wb": np.ascontiguousarray(wb),
        "wr": np.ascontiguousarray(wr),
        "peT": peT.astype(bfl),
        "id8b": np.eye(NH, dtype=bfl),
        "ones8": np.ones((NH, 1), np.float32),
        "fB": np.ascontiguousarray(fB),
    }
    in_maps = []
    for c in range(N_CORES):
        sl = slice(c * BPC, (c + 1) * BPC)
        m = dict(shared)
        m["drow"] = np.concatenate(
            [data_f32[sl].reshape(-1), idxl[sl]]
        ).reshape(1, -1).astype(bfl)
        m["fA"] = np.ascontiguousarray(np.concatenate(
            [pelT[:, sl], np.repeat(np.eye(NH, dtype=np.float32), DH, axis=0)],
            axis=1)).astype(np.float32)                      # [512, 12]
        m["fC"] = np.ascontiguousarray(np.concatenate(
            [np.broadcast_to(p[sl].astype(np.float32), (NH, BPC)),
             np.asarray(b2).reshape(NOUT, 1).astype(np.float32),
             iota8], axis=1)).astype(np.float32)             # [8, 1029]
        in_maps.append(m)
    return in_maps


def kernel(data, lengths, emb, Wq, bq, Wk, bk, Wv, bv, W1, b1, W2, b2):
    nc = _get_module()
    in_maps = make_in_maps(
        np.asarray(data), np.asarray(lengths), emb, Wq, bq, Wk, bk, Wv, bv,
        W1, b1, W2, b2,
    )
    res = run_bass_kernel_spmd(nc, in_maps, list(range(N_CORES)))
    out = np.concatenate(
        [res.results[c]["out"].reshape(BPC) for c in range(N_CORES)]
    )
    return out.astype(np.float32)


# revision 23
# speedup vs baseline: 1.0396x; 1.0096x over previous
"""Trainium2 Bass kernel for nn_Attention_module_52166672777937.

Data-parallel over batch across 8 NeuronCores (4 sequences per core).

Algorithmic restructuring (numerically validated against the reference):
the module only consumes the attention output at the LAST valid position
of each sequence (take_along_axis with lengths-1), and attention is
causal, so only ONE query row per sequence matters.  Consequences:

  * q is computed for a single position per sequence.
  * K is never materialized: scores = (qblk.T @ Wk) @ x.T, using
    associativity of the K projection with the score contraction.
  * softmax runs over [H=8, L] scores per sequence (no L x L matrix).
  * ctx = softmax(scores) @ V needs V = x @ Wv.T for all positions -- the
    dominant matmul, kept on TensorE at fp32r full rate.

Device layout: x is built in transposed [E, L] layout directly via a
one-hot matmul gather (onehot[c, l] = (data[l] == c), x.T = emb.T @
onehot + pe.T), which feeds both the score matmul and the V projection
without any transposes of large tensors.
"""

import math
import sys

import ml_dtypes
import numpy as np

sys.path.insert(0, "/opt/trn_rl_repo")

import concourse.bacc as bacc
import concourse.bass as bass
import concourse.mybir as mybir
import concourse.tile as tile
from concourse.bass_utils import run_bass_kernel_spmd

dt = mybir.dt
AF = mybir.ActivationFunctionType
ALU = mybir.AluOpType
PSUM = bass.MemorySpace.PSUM

N_CORES = 8
B, L = 32, 1000
LP = 1024                 # padded sequence length (2 x 512 column tiles)
TW = 512                  # column-tile width (max fp32 moving operand / PSUM bank)
NT = LP // TW             # column tiles per sequence
BPC = B // N_CORES        # sequences per core
NCH = 256                 # vocabulary
E = 512                   # embedding dim
D = 512                   # d_model
NH, DH = 8, 64            # heads
HS = 512                  # pred hidden size
NOUT = 8
NEG = -1.0e30
SCALE = 1.0 / math.sqrt(DH)


def _build():
    nc = bacc.Bacc(
        "TRN2", target_bir_lowering=False, debug=False, num_devices=N_CORES
    )

    f32 = dt.float32
    f32r = dt.float32r
    bf16 = dt.bfloat16
    # --- packed inputs (few wide DMAs instead of many narrow ones) ------
    # bf16s: data+idxlast row; emb; wqT|wk|wvT; peT; id8b
    d_drow = nc.dram_tensor("drow", [1, BPC * LP + BPC], bf16,
                            kind="ExternalInput")
    d_emb = nc.dram_tensor("emb", [NCH, E], bf16, kind="ExternalInput")
    d_wb = nc.dram_tensor("wb", [E, 3 * D], bf16, kind="ExternalInput")
    d_peT = nc.dram_tensor("peT", [E, LP], bf16, kind="ExternalInput")
    d_id8b = nc.dram_tensor("id8b", [NH, NH], bf16, kind="ExternalInput")
    # f32r: w1T|w2T ; ones8
    d_wr = nc.dram_tensor("wr", [D, HS + NOUT], f32r, kind="ExternalInput")
    d_ones8 = nc.dram_tensor("ones8", [NH, 1], f32r, kind="ExternalInput")
    # f32: pelastT|hmask (512-row); bq|b1|cvals (128-row); plast|b2|iota (8-row)
    d_fA = nc.dram_tensor("fA", [D, BPC + NH], f32, kind="ExternalInput")
    d_fB = nc.dram_tensor("fB", [128, 10], f32, kind="ExternalInput")
    d_fC = nc.dram_tensor("fC", [NH, BPC + 1 + LP], f32, kind="ExternalInput")
    d_out = nc.dram_tensor("out", [1, BPC], f32, kind="ExternalOutput")

    with tile.TileContext(nc) as tc:
        with (
            tc.tile_pool(name="const", bufs=1) as cp,
            tc.tile_pool(name="work", bufs=2) as wp,
            tc.tile_pool(name="psx", bufs=2, space=PSUM) as psx,
            tc.tile_pool(name="psv", bufs=2, space=PSUM) as psv,
            tc.tile_pool(name="pss", bufs=2, space=PSUM) as pss,
            tc.tile_pool(name="psc", bufs=2, space=PSUM) as psc,
        ):
            # ---------------- constant loads (packed) -------------------
            # issued in order of first use so compute starts early:
            # fB(cvals) -> db0 -> emb -> wq -> wk -> peT -> wv -> ...
            fB_sb = cp.tile([128, 10], f32, name="fB", tag="fB")
            nc.sync.dma_start(out=fB_sb[:], in_=d_fB[:])
            bq_sb = fB_sb[:, 0:4]
            b1_sb = fB_sb[:, 4:8]
            cvals_sb = fB_sb[:, 8:10]
            dbb = []
            for b in range(BPC):
                t = cp.tile([128, LP], bf16, name=f"db{b}", tag=f"db{b}")
                dbb.append(t)
            nc.sync.dma_start(
                out=dbb[0][:],
                in_=d_drow[:, 0:LP].to_broadcast((128, LP)),
            )
            embp_sb = cp.tile([128, 2, E], bf16, name="embp", tag="embp")
            nc.sync.dma_start(
                out=embp_sb[:],
                in_=d_emb[:].rearrange("(c p) n -> p c n", p=128),
            )
            emb_sb = [embp_sb[:, c, :] for c in range(2)]
            wqp_sb = cp.tile([128, 4, D], bf16, name="wqp", tag="wqp")
            nc.sync.dma_start(
                out=wqp_sb[:],
                in_=d_wb[:, 0:D].rearrange("(c p) n -> p c n", p=128),
            )
            wqT_sb = [wqp_sb[:, e, :] for e in range(4)]
            wkp_sb = cp.tile([128, 4, D], bf16, name="wkp", tag="wkp")
            nc.sync.dma_start(
                out=wkp_sb[:],
                in_=d_wb[:, D:2 * D].rearrange("(c p) n -> p c n", p=128),
            )
            wk_sb = [wkp_sb[:, c, :] for c in range(4)]
            peTp_sb = cp.tile([128, 4, LP], bf16, name="peTp", tag="peTp")
            nc.sync.dma_start(
                out=peTp_sb[:],
                in_=d_peT[:].rearrange("(c p) n -> p c n", p=128),
            )
            peT_sb = [peTp_sb[:, e, :] for e in range(4)]
            wvp_sb = cp.tile([128, 4, D], bf16, name="wvp", tag="wvp")
            nc.sync.dma_start(
                out=wvp_sb[:],
                in_=d_wb[:, 2 * D:3 * D].rearrange("(c p) n -> p c n", p=128),
            )
            wvT_sb = [wvp_sb[:, e, :] for e in range(4)]
            idxb_sb = cp.tile([128, BPC], bf16, name="idxb", tag="idxb")
            nc.sync.dma_start(
                out=idxb_sb[:],
                in_=d_drow[:, BPC * LP:].to_broadcast((128, BPC)),
            )
            fA_sb = cp.tile([128, 4, BPC + NH], f32, name="fA", tag="fA")
            nc.sync.dma_start(
                out=fA_sb[:], in_=d_fA[:].rearrange("(c p) n -> p c n", p=128)
            )
            pelT_sb = [fA_sb[:, m, 0:BPC] for m in range(4)]
            hmask_sb = [fA_sb[:, m, BPC:BPC + NH] for m in range(4)]
            fC_sb = cp.tile([NH, BPC + 1 + LP], f32, name="fC", tag="fC")
            nc.sync.dma_start(out=fC_sb[:], in_=d_fC[:])
            plast_sb = fC_sb[:, 0:BPC]
            b2_sb = fC_sb[:, BPC:BPC + 1]
            iota_sb = fC_sb[:, BPC + 1:]
            id8b_sb = cp.tile([NH, NH], bf16, name="id8b", tag="id8b")
            nc.sync.dma_start(out=id8b_sb[:], in_=d_id8b[:])
            for b in range(1, BPC):
                nc.sync.dma_start(
                    out=dbb[b][:],
                    in_=d_drow[:, b * LP:(b + 1) * LP].to_broadcast(
                        (128, LP)),
                )
            drow_sb = cp.tile([1, BPC * LP + BPC], bf16, name="drow",
                              tag="drow")
            nc.sync.dma_start(out=drow_sb[:], in_=d_drow[:])
            wr_sb = cp.tile([128, 4, HS + NOUT], f32r, name="wr", tag="wr")
            nc.sync.dma_start(
                out=wr_sb[:], in_=d_wr[:].rearrange("(c p) n -> p c n", p=128)
            )
            w1T_sb = [wr_sb[:, m, 0:HS] for m in range(4)]
            w2T_sb = [wr_sb[:, m, HS:HS + NOUT] for m in range(4)]
            ones8_sb = cp.tile([NH, 1], f32r, name="ones8", tag="ones8")
            nc.sync.dma_start(out=ones8_sb[:], in_=d_ones8[:])

            madd_sb = [None] * BPC

            def emit_gather(b, t):
                # one-hot + x.T tile for (sequence b, column tile t)
                oh = []
                for c in range(2):
                    o = wp.tile([128, TW], bf16, name=f"oh{b}_{t}_{c}",
                                tag="oh", bufs=6)
                    nc.vector.tensor_scalar(
                        o[:], dbb[b][:, t * TW:(t + 1) * TW],
                        cvals_sb[:, c:c + 1], None, ALU.is_equal,
                    )
                    oh.append(o)
                xT = []
                for e in range(4):
                    p = psx.tile([128, TW], f32, name=f"xtp{b}_{t}_{e}",
                                 tag="xtp")
                    for c in range(2):
                        nc.tensor.matmul(
                            p[:], (emb_sb[c][:, e * 128:(e + 1) * 128]),
                            (oh[c][:]), start=(c == 0), stop=(c == 1),
                        )
                    x = wp.tile([128, TW], bf16, name=f"xT{b}_{t}_{e}",
                                tag=f"xT{e}", bufs=3)
                    nc.vector.tensor_tensor(
                        x[:], p[:], peT_sb[e][:, t * TW:(t + 1) * TW],
                        ALU.add,
                    )
                    xT.append(x)
                return xT

            # first tile's gather depends only on the earliest DMAs; emit it
            # ahead of the serial q-prep chain so the PE queue head has work
            xT_first = emit_gather(0, 0)

            # ---------------- x_last gather -> q ------------------------
            ohl = []
            for c in range(2):
                t = cp.tile([128, BPC], bf16, name=f"ohl{c}", tag=f"ohl{c}")
                nc.vector.tensor_scalar(
                    t[:], idxb_sb[:], cvals_sb[:, c:c + 1], None, ALU.is_equal
                )
                ohl.append(t)
            # x_last.T [E, BPC] = emb.T @ onehot_last + pe_last.T
            xlast_sb = []
            for e in range(4):
                p = psx.tile([128, BPC], f32, name=f"xlp{e}", tag="xtp")
                for c in range(2):
                    nc.tensor.matmul(
                        p[:], (emb_sb[c][:, e * 128:(e + 1) * 128]),
                        (ohl[c][:]), start=(c == 0), stop=(c == 1),
                    )
                t = cp.tile([128, BPC], bf16, name=f"xlast{e}", tag=f"xlast{e}")
                nc.vector.tensor_tensor(t[:], p[:], pelT_sb[e][:], ALU.add)
                xlast_sb.append(t)
            # q.T [D, BPC] = Wq @ x_last.T + bq
            qT_sb = []
            for d in range(4):
                p = psv.tile([128, BPC], f32, name=f"qp{d}", tag="vp")
                for e in range(4):
                    nc.tensor.matmul(
                        p[:], (wqT_sb[e][:, d * 128:(d + 1) * 128]),
                        (xlast_sb[e][:]), start=(e == 0), stop=(e == 3),
                    )
                t = cp.tile([128, BPC], f32, name=f"qT{d}", tag=f"qT{d}")
                nc.vector.tensor_scalar(t[:], p[:], bq_sb[:, d:d + 1], None,
                                        ALU.add)
                qT_sb.append(t)

            # ---------------- main loop over sequences ------------------
            out_sb = cp.tile([1, BPC], f32, name="out_sb", tag="out_sb")
            ctxT_sb = [cp.tile([128, BPC], f32r, name=f"ctxT{m}", tag=f"ctxT{m}")
                       for m in range(4)]
            for b in range(BPC):
                # --- per-sequence qkvec = qblk.T @ Wk (K never formed) --
                qblk = []
                for d in range(4):
                    t = cp.tile([128, NH], bf16, name=f"qblk{b}_{d}",
                                tag=f"qblk{b}_{d}")
                    nc.vector.tensor_scalar(
                        t[:], hmask_sb[d][:], qT_sb[d][:, b:b + 1], None,
                        ALU.mult,
                    )
                    qblk.append(t)
                qkvp = pss.tile([NH, E], f32, name=f"qkvp{b}", tag="sp")
                for d in range(4):
                    nc.tensor.matmul(
                        qkvp[:], (qblk[d][:]), (wk_sb[d][:]),
                        start=(d == 0), stop=(d == 3),
                    )
                qkv_sb = wp.tile([NH, E], bf16, name=f"qkv{b}", tag="qkv",
                                 bufs=2)
                nc.vector.tensor_copy(qkv_sb[:], qkvp[:])
                qkvT = []
                for e in range(4):
                    tp = pss.tile([128, NH], bf16, name=f"qkvTp{b}_{e}",
                                  tag="sp")
                    nc.tensor.transpose(
                        tp[:], qkv_sb[:, e * 128:(e + 1) * 128], id8b_sb[:]
                    )
                    t = cp.tile([128, NH], bf16, name=f"qkvT{b}_{e}",
                                tag=f"qkvT{b}_{e}")
                    nc.vector.tensor_copy(t[:], tp[:])
                    qkvT.append(t)

                # --- attention over the sequence ------------------------
                # causal/validity mask, additive (0 valid / -1e30 invalid);
                # generated just-in-time so it doesn't block the first tile
                m8 = cp.tile([NH, LP], f32, name=f"madd{b}", tag=f"madd{b}")
                nc.vector.tensor_scalar(
                    m8[:], iota_sb[:], plast_sb[:, b:b + 1], NEG,
                    ALU.is_gt, ALU.mult,
                )
                madd_sb[b] = m8
                ctxp = psc.tile([NH, D], f32, name=f"ctx{b}", tag="cp")
                den_sb = wp.tile([NH, NT], f32, name=f"den{b}", tag="den",
                                 bufs=2)
                for t in range(NT):
                    xT = xT_first if (b == 0 and t == 0) else emit_gather(b, t)
                    # scores [NH, TW] = qkvec @ x.T
                    sp = pss.tile([NH, TW], f32, name=f"s{b}_{t}", tag="sp")
                    for e in range(4):
                        nc.tensor.matmul(
                            sp[:], (qkvT[e][:]), (xT[e][:]),
                            start=(e == 0), stop=(e == 3),
                        )
                    # masked scaled scores -> exp (+ row-sum for denom)
                    sm = wp.tile([NH, TW], f32, name=f"sm{b}_{t}", tag="sm",
                                 bufs=3)
                    nc.vector.scalar_tensor_tensor(
                        sm[:], sp[:], SCALE, madd_sb[b][:, t * TW:(t + 1) * TW],
                        ALU.mult, ALU.add,
                    )
                    ex = wp.tile([NH, TW], bf16, name=f"ex{b}_{t}", tag="ex",
                                 bufs=3)
                    nc.scalar.activation(
                        ex[:], sm[:], AF.Exp,
                        accum_out=den_sb[:, t:t + 1],
                    )
                    # V tile + ctx accumulation, one 128-row block at a time
                    for lc in range(4):
                        ap = pss.tile([128, NH], bf16, name=f"aTp{b}_{t}_{lc}",
                                      tag="sp")
                        nc.tensor.transpose(
                            ap[:], ex[:, lc * 128:(lc + 1) * 128], id8b_sb[:]
                        )
                        aT = wp.tile([128, NH], bf16, name=f"aT{b}_{t}_{lc}",
                                     tag="aT", bufs=8)
                        nc.scalar.copy(aT[:], ap[:])
                        vp = psv.tile([128, D], f32, name=f"vp{b}_{t}_{lc}",
                                      tag="vp")
                        for e in range(4):
                            nc.tensor.matmul(
                                vp[:],
                                (xT[e][:, lc * 128:(lc + 1) * 128]),
                                (wvT_sb[e][:]),
                                start=(e == 0), stop=(e == 3),
                            )
                        v = wp.tile([128, D], bf16, name=f"v{b}_{t}_{lc}",
                                    tag="v", bufs=4)
                        if lc % 2 == 0:
                            nc.vector.tensor_copy(v[:], vp[:])
                        else:
                            nc.scalar.copy(v[:], vp[:])
                        nc.tensor.matmul(
                            ctxp[:], (aT[:]), (v[:]),
                            start=(t == 0 and lc == 0),
                            stop=(t == NT - 1 and lc == 3),
                        )
                # normalize ctx rows by the masked softmax denominator
                dsum = wp.tile([NH, 1], f32, name=f"dsum{b}", tag="dsum",
                               bufs=2)
                nc.vector.reduce_sum(dsum[:], den_sb[:],
                                     axis=mybir.AxisListType.X)
                rec = wp.tile([NH, 1], f32, name=f"rec{b}", tag="rec", bufs=2)
                nc.vector.reciprocal(rec[:], dsum[:])
                ctx_sb = wp.tile([NH, D], bf16, name=f"ctxs{b}", tag="ctxs",
                                 bufs=2)
                nc.scalar.activation(ctx_sb[:], ctxp[:], AF.Copy,
                                     scale=rec[:])
                # extract block-diagonal -> ctx.T [D, BPC] column b
                for m in range(4):
                    tp = pss.tile([128, NH], bf16, name=f"ctp{b}_{m}", tag="sp")
                    nc.tensor.transpose(
                        tp[:], ctx_sb[:, m * 128:(m + 1) * 128], id8b_sb[:]
                    )
                    scr = wp.tile([128, NH], f32, name=f"scr{b}_{m}",
                                  tag="scr", bufs=2)
                    nc.vector.tensor_tensor(scr[:], tp[:], hmask_sb[m][:],
                                            ALU.mult)
                    with nc.allow_low_precision("fp32 accum, fp32r round"):
                        nc.vector.tensor_reduce(
                            ctxT_sb[m][:, b:b + 1], scr[:],
                            mybir.AxisListType.X, ALU.add,
                        )

            # ---------------- prediction head ---------------------------
            hT_sb = []
            for hc in range(4):
                p = psv.tile([128, BPC], f32, name=f"hp{hc}", tag="vp")
                for m in range(4):
                    nc.tensor.matmul(
                        p[:], (w1T_sb[m][:, hc * 128:(hc + 1) * 128]),
                        (ctxT_sb[m][:]), start=(m == 0), stop=(m == 3),
                    )
                t1 = wp.tile([128, BPC], f32, name=f"t1_{hc}", tag="t1",
                             bufs=2)
                nc.vector.tensor_scalar(t1[:], p[:], b1_sb[:, hc:hc + 1],
                                        None, ALU.add)
                ht = cp.tile([128, BPC], f32r, name=f"hT{hc}", tag=f"hT{hc}")
                nc.vector.scalar_tensor_tensor(
                    ht[:], t1[:], 0.01, t1[:], ALU.mult, ALU.max
                )
                hT_sb.append(ht)
            r2p = pss.tile([NOUT, BPC], f32, name="r2p", tag="sp")
            for hc in range(4):
                nc.tensor.matmul(
                    r2p[:], (w2T_sb[hc][:]), (hT_sb[hc][:]),
                    start=(hc == 0), stop=(hc == 3),
                )
            r_sb = cp.tile([NOUT, BPC], f32r, name="r_sb", tag="r_sb")
            nc.vector.tensor_scalar(r_sb[:], r2p[:], b2_sb[:], 0.0,
                                    ALU.add, ALU.max)
            mp = pss.tile([1, BPC], f32, name="mp", tag="sp")
            nc.tensor.matmul(mp[:], (ones8_sb[:]), (r_sb[:]))
            mt = cp.tile([1, BPC], f32, name="mt", tag="mt")
            nc.vector.tensor_scalar(mt[:], mp[:], 1.0 / NOUT, None, ALU.mult)
            nc.vector.scalar_tensor_tensor(
                out_sb[:], mt[:], 0.01, mt[:], ALU.mult, ALU.max
            )
            nc.sync.dma_start(out=d_out[:], in_=out_sb[:])

    nc.compile()
    return nc


_CACHE = {}


def _get_module():
    if "nc" not in _CACHE:
        _CACHE["nc"] = _build()
    return _CACHE["nc"]


def _pos_encoding():
    pos = np.arange(L, dtype=np.float32)[:, None]
    div = np.exp(
        np.arange(0, D, 2, dtype=np.float32) * (-math.log(10000.0) / D)
    )
    pe = np.zeros((L, D), np.float32)
    pe[:, 0::2] = np.sin(pos * div)
    pe[:, 1::2] = np.cos(pos * div)
    return pe


def make_in_maps(data, lengths, emb, Wq, bq, Wk, bk, Wv, bv, W1, b1, W2, b2):
    # the kernel folds the K-projection into the score contraction; a
    # nonzero bk would add a per-head constant q.bk_h to the scores, which
    # this build omits (bk is zero for this module).
    assert float(np.abs(np.asarray(bk)).max()) == 0.0
    # V eviction is a plain copy; nonzero bv would need a bias add there.
    assert float(np.abs(np.asarray(bv)).max()) == 0.0

    pe = _pos_encoding()                       # [L, D]
    peT = np.zeros((E, LP), np.float32)
    peT[:, :L] = pe.T

    dpad = np.zeros((B, LP), np.int64)
    dpad[:, :L] = data
    data_f32 = dpad.astype(np.float32)

    p = (np.asarray(lengths).astype(np.int64) - 1)          # [B]
    idxl = np.asarray(data)[np.arange(B), p].astype(np.float32)
    pelT = pe[p].astype(np.float32).T                       # [D, B]

    bfl = ml_dtypes.bfloat16
    wb = np.concatenate(
        [np.asarray(Wq).T, np.asarray(Wk), np.asarray(Wv).T], axis=1
    ).astype(bfl)                                            # [512, 1536]
    wr = np.concatenate(
        [np.asarray(W1).T, np.asarray(W2).T], axis=1
    ).astype(np.float32)                                     # [512, 520]
    fB = np.concatenate(
        [np.asarray(bq).reshape(4, 128).T,
         np.asarray(b1).reshape(4, 128).T,
         np.arange(256, dtype=np.float32).reshape(2, 128).T], axis=1
    ).astype(np.float32)                                     # [128, 10]
    iota8 = np.broadcast_to(np.arange(LP, dtype=np.float32), (NH, LP))
    shared = {
        "emb": np.ascontiguousarray(emb, dtype=bfl),
        "wb": np.ascontiguousarray(wb),
        "wr": np.ascontiguousarray(wr),
        "peT": peT.astype(bfl),
        "id8b": np.eye(NH, dtype=bfl),
        "ones8": np.ones((NH, 1), np.float32),
        "fB": np.ascontiguousarray(fB),
    }
    in_maps = []
    for c in range(N_CORES):
        sl = slice(c * BPC, (c + 1) * BPC)
        m = dict(shared)
        m["drow"] = np.concatenate(
            [data_f32[sl].reshape(-1), idxl[sl]]
        ).reshape(1, -1).astype(bfl)
        m["fA"] = np.ascontiguousarray(np.concatenate(
            [pelT[:, sl], np.repeat(np.eye(NH, dtype=np.float32), DH, axis=0)],
            axis=1)).astype(np.float32)                      # [512, 12]
        m["fC"] = np.ascontiguousarray(np.concatenate(
            [np.broadcast_to(p[sl].astype(np.float32), (NH, BPC)),
             np.asarray(b2).reshape(NOUT, 1).astype(np.float32),
             iota8], axis=1)).astype(np.float32)             # [8, 1029]
        in_maps.append(m)
    return in_maps


def kernel(data, lengths, emb, Wq, bq, Wk, bk, Wv, bv, W1, b1, W2, b2):
    nc = _get_module()
    in_maps = make_in_maps(
        np.asarray(data), np.asarray(lengths), emb, Wq, bq, Wk, bk, Wv, bv,
        W1, b1, W2, b2,
    )
    res = run_bass_kernel_spmd(nc, in_maps, list(range(N_CORES)))
    out = np.concatenate(
        [res.results[c]["out"].reshape(BPC) for c in range(N_CORES)]
    )
    return out.astype(np.float32)
